# revision 17
# baseline (speedup 1.0000x reference)
"""V7: input-specialized static gather schedule.

(was V6: native float32r data path — 1 cyc/row PE vs 4 for fp32.)

Ragged segment mean, B=2048 L=512 D=512 f32, 8 cores, batch-sharded.
Per core: 4 groups of 64 b's; per group the needed rows (concat over b of
seq[b, begin:end)) form a compacted stream gathered by dma_gather in calls
of <=1024 row indices (2 KiB/row descriptors over 16 DMA engines). Per
128-row tile, DVE builds a [128, 64] selection matrix in one tensor_scalar
(sel[k,j] = (colidx[k]==j) * 1/len) and TensorE accumulates
psum[64, 512] += sel.T @ gtile — output lands directly in [b, d] layout.

V7 change vs V6: the call structure (number of gather calls and each call's
index count) is computed from the actual begin/end at kernel() time and
baked into the compiled program as static constants, identical across all
8 cores (per-(core,group) row totals are equalized by the serpentine
length-balanced assignment, padded with a few junk row-0 indices that the
selection matrix masks out). This removes the V6 worst-case-sized schedule's
~28 empty gather calls (each cost a serial ~1.7us Pool-gen/DMA pipeline
bubble), the runtime count value_loads, and shrinks the index/selector
preload from ~2.5 MB to ~0.6 MB. The first call's indices load in their own
tiny DMA so the first gather's Q7 descriptor gen starts ~1.5us earlier.
TimelineSim on the compiled module: 258.8us (V6) -> 204.7us (V7) against a
192.9us pure-DMA-transfer floor at 360 GB/s (DMA engines 96% busy; the
drain after the last transfer is bounded by the trailing matmul/copy/DMA
semaphore chain and is insensitive to the final call's size).

Boot-NaN guard: the first NBUF calls (first use of each rotating gather
buffer) are forced to a full 1024 indices so every gather buffer byte is
written with finite data before any stale-region read (partial calls leave
tile tails holding the previous call's rows; sel=0 rows must be finite,
not boot NaN, because 0*NaN = NaN in the PE).
"""

import time

import numpy as np

from concourse import bacc
import concourse.mybir as mybir
import concourse.tile as tile
from concourse.bass_utils import run_bass_kernel_spmd

B, L, D = 2048, 512, 512
NCORES = 8
BL = B // NCORES  # 256
GB = 64  # b's per group (region = GB*L = 32768 rows, int16 idx max)
GRPS = BL // GB  # 4 groups per core
MAXC = 1024  # max indices per dma_gather call (>1024 wedges Q7)
NBUF = 4  # gather buffer depth

_CACHE = {}
LAST_RESULTS = None
LAST_SPMD = None
RACE_CHECK = True


def _plan_structure(lengths_asm):
    """Static per-group call sizes, shared by all cores.

    lengths_asm: [NCORES, BL] lengths in assignment order. Per group the
    call sizes must be identical across cores, so size to the max core's
    row count (the serpentine assignment keeps the spread tiny).
    """
    rows_cg = lengths_asm.reshape(NCORES, GRPS, GB).sum(axis=2)
    T16 = (rows_cg.max(axis=0) + 15) // 16 * 16
    callsz = []
    for g in range(GRPS):
        t = int(T16[g])
        if g == 0:
            t = max(t, NBUF * MAXC)  # boot guard: first NBUF calls full
        full, rem = divmod(t, MAXC)
        sz = [MAXC] * full + ([rem] if rem else [])
        callsz.append(tuple(sz))
    return tuple(callsz)


def _build_bass(callsz):
    nc = bacc.Bacc("TRN2", detect_race_conditions=RACE_CHECK)
    f32 = mybir.dt.float32
    f32r = mybir.dt.float32r
    sizes_all = [c for g in callsz for c in g]
    tot_idx_cols = sum(c // 16 for c in sizes_all)
    tot_tiles = sum((c + 127) // 128 for c in sizes_all)

    c0cols = callsz[0][0] // 16  # first call's idx columns, loaded separately

    seq = nc.dram_tensor("seq", [BL, L, D], f32r, kind="ExternalInput")
    # per-tile selection inputs: columns 2t = colidx, 2t+1 = w
    colw = nc.dram_tensor("colw", [128, tot_tiles * 2], f32, kind="ExternalInput")
    gidx0 = nc.dram_tensor("gidx0", [128, c0cols], mybir.dt.int16,
                           kind="ExternalInput")
    gidx = nc.dram_tensor("gidx", [128, tot_idx_cols - c0cols], mybir.dt.int16,
                          kind="ExternalInput")
    iotaf = nc.dram_tensor("iotaf", [128, GB], f32, kind="ExternalInput")
    outn = nc.dram_tensor("outn", [BL, D], f32, kind="ExternalOutput")

    rows = seq[:].rearrange("b l d -> (b l) d")  # [BL*L, D]

    with tile.TileContext(nc) as tc:
        with (
            tc.tile_pool(name="gpool", bufs=NBUF) as gpool,
            tc.tile_pool(name="selp", bufs=6) as selp,
            tc.tile_pool(name="constp", bufs=1) as constp,
            tc.tile_pool(name="psump", bufs=2, space="PSUM") as psump,
            tc.tile_pool(name="outp", bufs=2) as outp,
        ):
            # call-0 idx first and alone (16 KB): the first gather's Q7
            # descriptor gen can start ~1.5us earlier than if it waited on
            # the full index load
            idx0_sb = constp.tile([128, c0cols], mybir.dt.int16)
            nc.sync.dma_start(out=idx0_sb[:], in_=gidx0[:])
            idx_sb = constp.tile([128, tot_idx_cols - c0cols], mybir.dt.int16)
            nc.sync.dma_start(out=idx_sb[:], in_=gidx[:])
            iota_f = constp.tile([128, GB], f32)
            nc.sync.dma_start(out=iota_f[:], in_=iotaf[:])
            colw_sb = constp.tile([128, tot_tiles * 2], f32)
            nc.sync.dma_start(out=colw_sb[:], in_=colw[:])

            icol = 0
            tcol = 0
            for g, sizes in enumerate(callsz):
                psum = psump.tile([GB, D], f32, tag="ps", name="psum")
                nt_total = sum((c + 127) // 128 for c in sizes)
                t_in_g = 0
                for c in sizes:
                    nt = (c + 127) // 128
                    gtile = gpool.tile([128, 8 * D], f32r, tag="g", name="gtile")
                    if icol == 0:
                        isrc = idx0_sb[:, 0:c0cols]
                    else:
                        isrc = idx_sb[:, icol - c0cols : icol - c0cols + c // 16]
                    nc.gpsimd.dma_gather(
                        gtile[:].rearrange("p (c e) -> p c e", e=D)[:, :nt, :],
                        rows[g * GB * L : (g + 1) * GB * L, :],
                        isrc,
                        c,
                        c,
                        D,
                    )
                    icol += c // 16
                    for t in range(nt):
                        sel = selp.tile([128, GB], f32r, tag="sel", name="sel")
                        nc.vector.tensor_scalar(
                            out=sel[:],
                            in0=iota_f[:],
                            scalar1=colw_sb[:, 2 * tcol : 2 * tcol + 1],
                            scalar2=colw_sb[:, 2 * tcol + 1 : 2 * tcol + 2],
                            op0=mybir.AluOpType.is_equal,
                            op1=mybir.AluOpType.mult,
                        )
                        nc.tensor.matmul(
                            out=psum[:],
                            lhsT=sel[:],
                            rhs=gtile[:, t * D : (t + 1) * D],
                            start=(t_in_g == 0),
                            stop=(t_in_g == nt_total - 1),
                        )
                        tcol += 1
                        t_in_g += 1
                out_sb = outp.tile([GB, D], f32, tag="out", name="out_sb")
                nc.vector.tensor_copy(out=out_sb[:], in_=psum[:])
                nc.sync.dma_start(
                    out=outn[g * GB : (g + 1) * GB, :], in_=out_sb[:]
                )
    nc.compile()
    return nc


def _get_bass(callsz):
    if callsz not in _CACHE:
        _CACHE[callsz] = _build_bass(callsz)
    return _CACHE[callsz]


def _host_prep(begin_c, end_c, callsz):
    """Per-core compacted gather indices + per-tile col/w selectors."""
    length = (end_c - begin_c).astype(np.int64)
    w_b = 1.0 / length.astype(np.float32)
    sizes_all = [c for g in callsz for c in g]
    tot_idx = sum(sizes_all)
    tot_tiles = sum((c + 127) // 128 for c in sizes_all)
    idx_all = np.zeros(tot_idx, dtype=np.int64)  # pad = row 0 (finite, sel 0)
    colidx = np.full((tot_tiles, 128), -1.0, dtype=np.float32)
    wcol = np.zeros((tot_tiles, 128), dtype=np.float32)
    iofs = 0
    tofs = 0
    for g, sizes in enumerate(callsz):
        bs = np.arange(g * GB, (g + 1) * GB)
        lens = length[bs]
        n_rows = int(lens.sum())
        assert n_rows <= sum(sizes), (g, n_rows, sizes)
        slots = np.repeat(np.arange(GB), lens)
        ls = np.concatenate(
            [np.arange(begin_c[b], end_c[b]) for b in bs]
        )
        ridx = slots * L + ls  # row index within group region
        svals = slots.astype(np.float32)
        wvals = w_b[bs][slots]
        pos = 0
        for c in sizes:
            nt = (c + 127) // 128
            take = min(c, max(n_rows - pos, 0))
            if take > 0:
                seg = slice(pos, pos + take)
                ar = np.arange(take)
                idx_all[iofs : iofs + take] = ridx[seg]
                colidx[tofs + ar // 128, ar % 128] = svals[seg]
                wcol[tofs + ar // 128, ar % 128] = wvals[seg]
            pos += c
            iofs += c
            tofs += nt
    assert 0 <= idx_all.min() and idx_all.max() < GB * L
    idx16 = idx_all.astype(np.int16).reshape(-1, 16).T  # [16, tot/16]
    gidx = np.ascontiguousarray(np.tile(idx16, (8, 1)))  # [128, tot/16]
    colw = np.empty((128, tot_tiles * 2), dtype=np.float32)
    colw[:, 0::2] = colidx.T
    colw[:, 1::2] = wcol.T
    c0cols = callsz[0][0] // 16
    return (
        np.ascontiguousarray(colw),
        np.ascontiguousarray(gidx[:, :c0cols]),
        np.ascontiguousarray(gidx[:, c0cols:]),
    )


def _balanced_assignment(length):
    """Assign b's to cores, serpentine over descending length, so per-core
    (and per-core-per-group) total gathered rows are near-equal."""
    order = np.argsort(-length, kind="stable")
    asm = np.empty((NCORES, BL), dtype=np.int64)
    for r in range(BL):
        cores = range(NCORES) if r % 2 == 0 else range(NCORES - 1, -1, -1)
        for j, c in enumerate(cores):
            asm[c, r] = order[r * NCORES + j]
    return asm


def kernel(seq, begin, end):
    global LAST_RESULTS, LAST_SPMD
    seq = np.ascontiguousarray(np.asarray(seq, dtype=np.float32))
    begin_i = np.asarray(begin).astype(np.int64)
    end_i = np.asarray(end).astype(np.int64)
    length = end_i - begin_i
    asm = _balanced_assignment(length)
    callsz = _plan_structure(length[asm])

    nc = _get_bass(callsz)
    iota_np = np.broadcast_to(
        np.arange(GB, dtype=np.float32)[None, :], (128, GB)
    ).copy()
    in_maps = []
    for c in range(NCORES):
        bs = asm[c]
        colw, gidx0, gidxa = _host_prep(begin_i[bs], end_i[bs], callsz)
        in_maps.append(
            {"seq": seq[bs], "colw": colw, "gidx0": gidx0, "gidx": gidxa,
             "iotaf": iota_np}
        )

    LAST_SPMD = (nc, in_maps)
    # the axon-tunneled devices occasionally report a transient
    # NRT_EXEC_UNIT_UNRECOVERABLE; a fresh attempt recovers
    last_exc = None
    for attempt in range(4):
        try:
            LAST_RESULTS = run_bass_kernel_spmd(
                nc, in_maps, core_ids=list(range(NCORES))
            )
            break
        except Exception as e:  # noqa: BLE001
            last_exc = e
            time.sleep(10.0 * (attempt + 1))
    else:
        raise last_exc
    out = np.empty((B, D), dtype=np.float32)
    for c in range(NCORES):
        out[asm[c]] = LAST_RESULTS.results[c]["outn"]
    return out


# revision 30
# speedup vs baseline: 1.0002x; 1.0002x over previous
"""V7: input-specialized static gather schedule.

(was V6: native float32r data path — 1 cyc/row PE vs 4 for fp32.)

Ragged segment mean, B=2048 L=512 D=512 f32, 8 cores, batch-sharded.
Per core: 4 groups of 64 b's; per group the needed rows (concat over b of
seq[b, begin:end)) form a compacted stream gathered by dma_gather in calls
of <=1024 row indices (2 KiB/row descriptors over 16 DMA engines). Per
128-row tile, DVE builds a [128, 64] selection matrix in one tensor_scalar
(sel[k,j] = (colidx[k]==j) * 1/len) and TensorE accumulates
psum[64, 512] += sel.T @ gtile — output lands directly in [b, d] layout.

V7 change vs V6: the call structure (number of gather calls and each call's
index count) is computed from the actual begin/end at kernel() time and
baked into the compiled program as static constants, identical across all
8 cores (per-(core,group) row totals are equalized by the serpentine
length-balanced assignment, padded with a few junk row-0 indices that the
selection matrix masks out). This removes the V6 worst-case-sized schedule's
~28 empty gather calls (each cost a serial ~1.7us Pool-gen/DMA pipeline
bubble), the runtime count value_loads, and shrinks the index/selector
preload from ~2.5 MB to ~0.6 MB. The first call's indices load in their own
tiny DMA so the first gather's Q7 descriptor gen starts ~1.5us earlier.
TimelineSim on the compiled module: 258.8us (V6) -> 204.7us (V7) against a
192.9us pure-DMA-transfer floor at 360 GB/s (DMA engines 96% busy; the
drain after the last transfer is bounded by the trailing matmul/copy/DMA
semaphore chain and is insensitive to the final call's size).

Boot-NaN guard: the first NBUF calls (first use of each rotating gather
buffer) are forced to a full 1024 indices so every gather buffer byte is
written with finite data before any stale-region read (partial calls leave
tile tails holding the previous call's rows; sel=0 rows must be finite,
not boot NaN, because 0*NaN = NaN in the PE).
"""

import time

import numpy as np

from concourse import bacc
import concourse.mybir as mybir
import concourse.tile as tile
from concourse.bass_utils import run_bass_kernel_spmd

B, L, D = 2048, 512, 512
NCORES = 8
BL = B // NCORES  # 256
GB = 64  # b's per group (region = GB*L = 32768 rows, int16 idx max)
GRPS = BL // GB  # 4 groups per core
MAXC = 1024  # max indices per dma_gather call (>1024 wedges Q7)
NBUF = 4  # gather buffer depth
# Ramp prefix for group 0 (small first calls in dedicated buffers to start
# the DMA stream earlier). Measured NET-NEGATIVE in TimelineSim: the start
# gains ~350ns but the shifted matmul burst phases interact with the PE
# p-state clock ramp (788ns/tile cold vs 213ns warm) and grow the terminal
# PE backlog by ~4.4us. Kept as a knob, disabled.
RAMP = ()

_CACHE = {}
LAST_RESULTS = None
LAST_SPMD = None
RACE_CHECK = True


def _plan_structure(lengths_asm):
    """Static per-group call sizes, shared by all cores.

    lengths_asm: [NCORES, BL] lengths in assignment order. Per group the
    call sizes must be identical across cores, so size to the max core's
    row count (the serpentine assignment keeps the spread tiny).
    """
    rows_cg = lengths_asm.reshape(NCORES, GRPS, GB).sum(axis=2)
    T16 = (rows_cg.max(axis=0) + 15) // 16 * 16
    callsz = []
    for g in range(GRPS):
        t = int(T16[g])
        if g == 0:
            # ramp prefix + boot guard: first NBUF gpool calls full
            t = max(t, sum(RAMP) + NBUF * MAXC)
            head = t - sum(RAMP)
            full, rem = divmod(head, MAXC)
            sz = list(RAMP) + [MAXC] * full + ([rem] if rem else [])
        else:
            full, rem = divmod(t, MAXC)
            sz = [MAXC] * full + ([rem] if rem else [])
        callsz.append(tuple(sz))
    return tuple(callsz)


def _build_bass(callsz):
    nc = bacc.Bacc("TRN2", detect_race_conditions=RACE_CHECK)
    f32 = mybir.dt.float32
    f32r = mybir.dt.float32r
    sizes_all = [c for g in callsz for c in g]
    tot_idx_cols = sum(c // 16 for c in sizes_all)
    tot_tiles = sum((c + 127) // 128 for c in sizes_all)

    nramp = len(RAMP) if callsz[0][: len(RAMP)] == RAMP else 0
    # idx of the ramp calls (or just call 0) load separately and first
    c0cols = sum(callsz[0][: max(nramp, 1)]) // 16

    seq = nc.dram_tensor("seq", [BL, L, D], f32r, kind="ExternalInput")
    # per-tile selection input: column t = colidx of tile t (the 1/len
    # weight is applied once per group at psum copy-out, not per row)
    colw = nc.dram_tensor("colw", [128, tot_tiles], f32, kind="ExternalInput")
    gidx0 = nc.dram_tensor("gidx0", [128, c0cols], mybir.dt.int16,
                           kind="ExternalInput")
    gidx = nc.dram_tensor("gidx", [128, tot_idx_cols - c0cols], mybir.dt.int16,
                          kind="ExternalInput")
    # cols [0:GB] = iota row, cols [GB:GB+GRPS] = per-slot 1/len per group
    iotaf = nc.dram_tensor("iotaf", [128, GB + GRPS], f32, kind="ExternalInput")
    outn = nc.dram_tensor("outn", [BL, D], f32, kind="ExternalOutput")

    rows = seq[:].rearrange("b l d -> (b l) d")  # [BL*L, D]

    with tile.TileContext(nc) as tc:
        with (
            tc.tile_pool(name="gpool", bufs=NBUF) as gpool,
            tc.tile_pool(name="selp", bufs=6) as selp,
            tc.tile_pool(name="constp", bufs=1) as constp,
            tc.tile_pool(name="psump", bufs=2, space="PSUM") as psump,
            tc.tile_pool(name="outp", bufs=2) as outp,
        ):
            # call-0 idx first and alone (16 KB): the first gather's Q7
            # descriptor gen can start ~1.5us earlier than if it waited on
            # the full index load
            idx0_sb = constp.tile([128, c0cols], mybir.dt.int16)
            nc.sync.dma_start(out=idx0_sb[:], in_=gidx0[:])
            idx_sb = constp.tile([128, tot_idx_cols - c0cols], mybir.dt.int16)
            nc.sync.dma_start(out=idx_sb[:], in_=gidx[:])
            iota_f = constp.tile([128, GB + GRPS], f32)
            nc.sync.dma_start(out=iota_f[:], in_=iotaf[:])
            colw_sb = constp.tile([128, tot_tiles], f32)
            nc.sync.dma_start(out=colw_sb[:], in_=colw[:])

            # dedicated one-shot buffers for the ramp calls (never recycled,
            # so their unwritten tile tails are never read)
            ramp_tiles = [
                constp.tile([128, (r // 128) * D], f32r, name=f"ramp{i}")
                for i, r in enumerate(callsz[0][:nramp])
            ]

            icol = 0
            tcol = 0
            ncall = 0
            for g, sizes in enumerate(callsz):
                psum = psump.tile([GB, D], f32, tag="ps", name="psum")
                nt_total = sum((c + 127) // 128 for c in sizes)
                t_in_g = 0
                for c in sizes:
                    nt = (c + 127) // 128
                    if ncall < nramp:
                        gtile = ramp_tiles[ncall]
                    else:
                        gtile = gpool.tile([128, 8 * D], f32r, tag="g",
                                           name="gtile")
                    ncall += 1
                    if icol < c0cols:
                        isrc = idx0_sb[:, icol : icol + c // 16]
                    else:
                        isrc = idx_sb[:, icol - c0cols : icol - c0cols + c // 16]
                    nc.gpsimd.dma_gather(
                        gtile[:].rearrange("p (c e) -> p c e", e=D)[:, :nt, :],
                        rows[g * GB * L : (g + 1) * GB * L, :],
                        isrc,
                        c,
                        c,
                        D,
                    )
                    icol += c // 16
                    for t in range(nt):
                        sel = selp.tile([128, GB], f32r, tag="sel", name="sel")
                        nc.vector.tensor_scalar(
                            out=sel[:],
                            in0=iota_f[:, 0:GB],
                            scalar1=colw_sb[:, tcol : tcol + 1],
                            scalar2=None,
                            op0=mybir.AluOpType.is_equal,
                        )
                        nc.tensor.matmul(
                            out=psum[:],
                            lhsT=sel[:],
                            rhs=gtile[:, t * D : (t + 1) * D],
                            start=(t_in_g == 0),
                            stop=(t_in_g == nt_total - 1),
                        )
                        tcol += 1
                        t_in_g += 1
                out_sb = outp.tile([GB, D], f32, tag="out", name="out_sb")
                nc.vector.tensor_scalar(
                    out=out_sb[:],
                    in0=psum[:],
                    scalar1=iota_f[0:GB, GB + g : GB + g + 1],
                    scalar2=None,
                    op0=mybir.AluOpType.mult,
                )
                nc.sync.dma_start(
                    out=outn[g * GB : (g + 1) * GB, :], in_=out_sb[:]
                )
    nc.compile()
    return nc


def _get_bass(callsz):
    if callsz not in _CACHE:
        _CACHE[callsz] = _build_bass(callsz)
    return _CACHE[callsz]


def _host_prep(begin_c, end_c, callsz):
    """Per-core compacted gather indices + per-tile colidx + per-slot 1/len."""
    length = (end_c - begin_c).astype(np.int64)
    w_b = 1.0 / length.astype(np.float32)
    sizes_all = [c for g in callsz for c in g]
    tot_idx = sum(sizes_all)
    tot_tiles = sum((c + 127) // 128 for c in sizes_all)
    idx_all = np.zeros(tot_idx, dtype=np.int64)  # pad = row 0 (finite, sel 0)
    colidx = np.full((tot_tiles, 128), -1.0, dtype=np.float32)
    iotaw = np.zeros((128, GB + GRPS), dtype=np.float32)
    iotaw[:, 0:GB] = np.arange(GB, dtype=np.float32)[None, :]
    iofs = 0
    tofs = 0
    for g, sizes in enumerate(callsz):
        bs = np.arange(g * GB, (g + 1) * GB)
        lens = length[bs]
        n_rows = int(lens.sum())
        assert n_rows <= sum(sizes), (g, n_rows, sizes)
        iotaw[0:GB, GB + g] = w_b[bs]
        slots = np.repeat(np.arange(GB), lens)
        ls = np.concatenate(
            [np.arange(begin_c[b], end_c[b]) for b in bs]
        )
        ridx = slots * L + ls  # row index within group region
        svals = slots.astype(np.float32)
        pos = 0
        for c in sizes:
            nt = (c + 127) // 128
            take = min(c, max(n_rows - pos, 0))
            if take > 0:
                seg = slice(pos, pos + take)
                ar = np.arange(take)
                idx_all[iofs : iofs + take] = ridx[seg]
                colidx[tofs + ar // 128, ar % 128] = svals[seg]
            pos += c
            iofs += c
            tofs += nt
    assert 0 <= idx_all.min() and idx_all.max() < GB * L
    idx16 = idx_all.astype(np.int16).reshape(-1, 16).T  # [16, tot/16]
    gidx = np.ascontiguousarray(np.tile(idx16, (8, 1)))  # [128, tot/16]
    c0cols = callsz[0][0] // 16
    return (
        np.ascontiguousarray(colidx.T),
        np.ascontiguousarray(gidx[:, :c0cols]),
        np.ascontiguousarray(gidx[:, c0cols:]),
        iotaw,
    )


def _balanced_assignment(length):
    """Assign b's to cores, serpentine over descending length, so per-core
    (and per-core-per-group) total gathered rows are near-equal."""
    order = np.argsort(-length, kind="stable")
    asm = np.empty((NCORES, BL), dtype=np.int64)
    for r in range(BL):
        cores = range(NCORES) if r % 2 == 0 else range(NCORES - 1, -1, -1)
        for j, c in enumerate(cores):
            asm[c, r] = order[r * NCORES + j]
    return asm


def kernel(seq, begin, end):
    global LAST_RESULTS, LAST_SPMD
    seq = np.ascontiguousarray(np.asarray(seq, dtype=np.float32))
    begin_i = np.asarray(begin).astype(np.int64)
    end_i = np.asarray(end).astype(np.int64)
    length = end_i - begin_i
    asm = _balanced_assignment(length)
    callsz = _plan_structure(length[asm])

    nc = _get_bass(callsz)
    in_maps = []
    for c in range(NCORES):
        bs = asm[c]
        colw, gidx0, gidxa, iotaw = _host_prep(begin_i[bs], end_i[bs], callsz)
        in_maps.append(
            {"seq": seq[bs], "colw": colw, "gidx0": gidx0, "gidx": gidxa,
             "iotaf": iotaw}
        )

    LAST_SPMD = (nc, in_maps)
    # the axon-tunneled devices occasionally report a transient
    # NRT_EXEC_UNIT_UNRECOVERABLE; a fresh attempt recovers
    last_exc = None
    for attempt in range(4):
        try:
            LAST_RESULTS = run_bass_kernel_spmd(
                nc, in_maps, core_ids=list(range(NCORES))
            )
            break
        except Exception as e:  # noqa: BLE001
            last_exc = e
            time.sleep(10.0 * (attempt + 1))
    else:
        raise last_exc
    out = np.empty((B, D), dtype=np.float32)
    for c in range(NCORES):
        out[asm[c]] = LAST_RESULTS.results[c]["outn"]
    return out
